# revision 29
# baseline (speedup 1.0000x reference)
"""BinaryWeightConv2d on Trainium2 — 8-core data-parallel over batch.

Reference computation (fp32):
    scale = clip(mean|w| over (in,kh,kw), 1e-8)          # per out-channel
    bw    = sign(w) * scale
    out   = conv2d(x, bw, stride 1, pad 1) + bias
    y     = ternary(out): 1 if out > 0.5, -1 if out < -0.5, else 0

Kernel strategy (mode f32r+j+ys+sb+ae+sp2+y8+wu, ~110-130us on 8 cores vs
357us fp16-pair baseline; cool-state pairs repeatedly measure 107-115us,
matching TimelineSim's 109.8us prediction):
  - Shard the batch (32) over 8 cores, 4 images each; replicate the tiny
    binarized weights (per the data-parallel sharding hint).
  - Device: conv = 9 shifted-window matmuls (3x3 taps) accumulating in PSUM;
    contraction over C=128 = the partition dim.  x is host-padded to 58x58
    per image so every tap window is one contiguous SBUF slice.  PSUM tile =
    8 output rows x 58 = 464 columns (one bank).
  - Matmul dtype float32r: 1 PE cycle/column when free dim >= 256 (same rate
    as fp16) at ~10-bit mantissa precision, so 9 matmuls/tile instead of the
    fp16 hi/lo pair's 18.  Ternary rel err 8.7e-3 (deterministic, gate 2e-2).
    PE streaming floor 504 MMs x 464 cols ~ 97us/core; measured ~206ns/MM
    incl. the per-MM LDWEIGHTS walrus always emits (no dedup - verified in
    the sunda disasm; grouped weight reuse does NOT help, measured).
  - Epilogue on the otherwise-idle ACT engine ("ae"): per-out-channel scale s
    and biases b-+0.5 folded into two Sign activations,
    y2 = Sign(s*S + b - 0.5) + Sign(s*S + b + 0.5) in {-2,0,2};
    one DVE add combines them; host multiplies by 0.5.  Frees the DVE
    (was 51% busy) and shortens the post-matmul tail.
  - "sb": fine-grained leading DMA chunks (w 3 taps -> 6 -> rest on the ACT
    HWDGE ring; x rows 0-9 first on the sync ring) so the first matmul
    starts ~2us in instead of ~11us.
  - "sp2": the last tile's epilogue runs as two half-tiles to shorten the
    final ACT->DVE->DMA chain.
  - "y8": y stored as fp8e4 (+-2,0 exact) halving output DMA volume.
  - "wu": 64 tiny const matmuls during the leading DMA wait keep the PE busy
    so the HAM clock gate un-throttles (1.2 -> 2.4 GHz) before real work.
  - Outputs are full padded [C, 464] tiles (contiguous DMA segments); host
    strips the 2 junk columns per 58-wide row and scales by 0.5.
  - Timing method (test.py): wall-clock differencing of For_i(R) repeats at
    R=64 vs 4096, wall-floor estimator over 5 cool-ish pairs (30s idles —
    sustained load triggers a P0 downclock that inflates per-iter time ~30%).
"""

import os
import numpy as np

N, C, H, W = 32, 128, 56, 56
O = 256
NCORES = 8
NPC = N // NCORES           # images per core
HP, WP = H + 2, W + 2       # padded spatial
IMG = HP * WP               # 3364
XCOLS = NPC * IMG           # 13456
XCOLS_PAD = XCOLS + 64      # slack: the last tap of the last tile overreads 1
RB = 8                      # output rows per PSUM tile
NT = RB * WP                # 464 = PSUM tile free size (<= 512 bank limit)
NBLK = H // RB              # 7 row blocks
TAPS = [(kh, kw) for kh in range(3) for kw in range(3)]

MODE = os.environ.get("BWC_MODE", "f32r+j+ys+sb+ae+sp2+y8+wu")

_prog_cache = {}


def _build(mode, repeat=1):
    import concourse.tile as tile
    from concourse import mybir, bacc
    from contextlib import ExitStack

    dt = mybir.dt
    nc = bacc.Bacc()

    parts = mode.split("+")
    base, flags = parts[0], set(parts[1:])
    jpad = "j" in flags
    ydt_bf = "h" in flags       # store ternary output as bf16 (host converts)
    dmaless = "dl" in flags     # timing probe: ~zero output DMA volume
    act_out = "a" in flags      # issue output stores on the ACT HWDGE ring
    obufs = 16 if "o16" in flags else 6
    ysplit = "ys" in flags      # dedicated deep pool for DMA-held y tiles
    sb = "sb" in flags          # fine-grained start chunks + per-oc w layout
    ae = "ae" in flags          # ACT Sign-pair epilogue ({-2,0,2} out, host /2)
    il2 = "il2" in flags        # interleave matmuls across pairs of tiles
    sp2 = "sp2" in flags        # split last tile's epilogue into halves (tail)
    y8 = "y8" in flags          # store y as fp8e4 (exact for {-2,0,2})
    a2 = "a2" in flags          # alternate y stores across SP/ACT HWDGE rings
    eb = "eb" in flags          # balance epilogue: alternate ACT-pair / DVE-pair
    wu = "wu" in flags          # HAM warm-up: dummy matmuls during DMA wait

    if base == "f16p":
        mm_dt, np_mm = dt.float16, np.float16
    elif base == "f32r":
        mm_dt, np_mm = dt.float32r, np.float32
    elif base == "f32":
        mm_dt, np_mm = dt.float32, np.float32
    else:
        raise ValueError(mode)
    pair = base == "f16p"

    xh_d = nc.declare_dram_parameter("xh", [C, XCOLS_PAD], mm_dt, isOutput=False)
    xl_d = (nc.declare_dram_parameter("xl", [C, XCOLS_PAD], mm_dt, isOutput=False)
            if pair else None)
    sw_d = nc.declare_dram_parameter("sw", [C, 9 * O], mm_dt, isOutput=False)
    thr_d = nc.declare_dram_parameter("thr", [C, 8 if ae else 4], dt.float32,
                                      isOutput=False)
    out_dt = dt.float8e4 if y8 else (dt.bfloat16 if (ydt_bf or ae) else dt.float32)
    if jpad:
        out_d = nc.declare_dram_parameter("out", [NPC, 2, NBLK, C, NT],
                                          out_dt, isOutput=True)
    else:
        out_d = nc.declare_dram_parameter("out", [2, C, NPC, H, W],
                                          out_dt, isOutput=True)

    with tile.TileContext(nc) as tc, ExitStack() as ctx:
        inp = ctx.enter_context(tc.tile_pool(name="inp", bufs=2))
        outp = ctx.enter_context(tc.tile_pool(name="outp", bufs=4 if ysplit else obufs))
        ypool = (ctx.enter_context(tc.tile_pool(name="ypool", bufs=24))
                 if ysplit else outp)
        psum = ctx.enter_context(tc.tile_pool(name="psum", bufs=8, space="PSUM"))

        def body():
            t_w = inp.tile([C, 9 * O], mm_dt, tag="w")
            t_thr = inp.tile([C, 8 if ae else 4], dt.float32, tag="thr")
            t_xh = inp.tile([C, XCOLS_PAD], mm_dt, tag="xh")
            t_xl = (inp.tile([C, XCOLS_PAD], mm_dt, tag="xl", name="t_xl")
                    if pair else None)

            if sb:
                # fine-grained leading chunks so PE starts ~2us in;
                # w/thr ride the ACT HWDGE ring so their latency overlaps x's
                xbounds = [0, 10 * WP, 18 * WP, 34 * WP,
                           IMG, 2 * IMG, 3 * IMG, XCOLS_PAD]
                nc.sync.dma_start(t_xh[:, :xbounds[1]], xh_d[:, :xbounds[1]])
                nc.scalar.dma_start(t_w[:, :3 * C], sw_d[:, :3 * C])
                nc.scalar.dma_start(t_thr[:], thr_d[:])
                nc.sync.dma_start(t_xh[:, xbounds[1]:xbounds[2]],
                                  xh_d[:, xbounds[1]:xbounds[2]])
                nc.scalar.dma_start(t_w[:, 3 * C:9 * C], sw_d[:, 3 * C:9 * C])
                nc.scalar.dma_start(t_w[:, 9 * C:], sw_d[:, 9 * C:])
                for i in range(2, 7):
                    lo, hi = xbounds[i], xbounds[i + 1]
                    nc.sync.dma_start(t_xh[:, lo:hi], xh_d[:, lo:hi])
            else:
                nc.sync.dma_start(t_w[:], sw_d[:])
                nc.sync.dma_start(t_thr[:], thr_d[:])
                # chunked x loads (per image) so compute starts after chunk 0
                bounds = [0, IMG, 2 * IMG, 3 * IMG, XCOLS_PAD]
                for i in range(4):
                    lo, hi = bounds[i], bounds[i + 1]
                    nc.sync.dma_start(t_xh[:, lo:hi], xh_d[:, lo:hi])
                    if pair:
                        nc.sync.dma_start(t_xl[:, lo:hi], xl_d[:, lo:hi])

            if wu:
                # keep the PE busy through the leading DMA wait so HAM
                # un-throttles (1.2 -> 2.4 GHz) before the first real matmul
                zc = nc.const_aps.tensor(0.0, (C, 1), dt.float32)
                wpt = psum.tile([C, 512], dt.float32, tag="pt")
                for _ in range(64):
                    nc.tensor.matmul(wpt[:1, :1], zc, zc, start=True, stop=True)

            nmm = 18 if pair else 9

            def wslice(oc, t):
                if sb:
                    return t_w[:, oc * (9 * C) + t * C: oc * (9 * C) + (t + 1) * C]
                return t_w[:, t * O + oc * C: t * O + oc * C + C]

            def emit_mms(group):
                # group: list of (n, oc, j, pt); taps outer, tiles inner
                for t, (kh, kw) in enumerate(TAPS):
                    for (n, oc, j, pt) in group:
                        base_off = n * IMG + (j * RB + kh) * WP + kw
                        nc.tensor.matmul(pt, wslice(oc, t),
                                         t_xh[:, base_off:base_off + NT],
                                         start=(t == 0),
                                         stop=(t == 8 and not pair))
                        if pair:
                            nc.tensor.matmul(pt, wslice(oc, t),
                                             t_xl[:, base_off:base_off + NT],
                                             start=False, stop=(t == 8))

            def epilogue(n, oc, j, pt):
                use_act = ae
                if ae and eb and ((n * 2 + oc) * NBLK + j) % 2 == 1:
                    use_act = False  # DVE path for odd tiles ({-1,0,1} out)
                if use_act:
                    s_ap = t_thr[:, 3 * oc + 0:3 * oc + 1]
                    bm_ap = t_thr[:, 3 * oc + 1:3 * oc + 2]
                    bp_ap = t_thr[:, 3 * oc + 2:3 * oc + 3]
                    last = (n == NPC - 1 and oc == 1 and j == NBLK - 1)
                    if sp2 and last and jpad and not dmaless:
                        # halve the tail chain: ACT->DVE->DMA per half-tile
                        HNT = NT // 2
                        for h in range(2):
                            sl = slice(h * HNT, (h + 1) * HNT)
                            a0 = outp.tile([C, HNT], dt.bfloat16, tag="a0h")
                            nc.scalar.activation(
                                a0[:], pt[:, sl],
                                mybir.ActivationFunctionType.Sign,
                                bias=bm_ap, scale=s_ap)
                            a1 = outp.tile([C, HNT], dt.bfloat16, tag="a1h")
                            nc.scalar.activation(
                                a1[:], pt[:, sl],
                                mybir.ActivationFunctionType.Sign,
                                bias=bp_ap, scale=s_ap)
                            y = ypool.tile([C, HNT], out_dt, tag="yh")
                            nc.vector.tensor_tensor(y[:], a0[:], a1[:],
                                                    mybir.AluOpType.add)
                            nc.sync.dma_start(out_d[n, oc, j][:, sl], y[:])
                        return
                    a0 = outp.tile([C, NT], dt.bfloat16, tag="a0")
                    nc.scalar.activation(a0[:], pt,
                                         mybir.ActivationFunctionType.Sign,
                                         bias=bm_ap, scale=s_ap)
                    a1 = outp.tile([C, NT], dt.bfloat16, tag="a1")
                    nc.scalar.activation(a1[:], pt,
                                         mybir.ActivationFunctionType.Sign,
                                         bias=bp_ap, scale=s_ap)
                    y = ypool.tile([C, NT], out_dt, tag="y")
                    nc.vector.tensor_tensor(y[:], a0[:], a1[:],
                                            mybir.AluOpType.add)
                else:
                    hi_ap = t_thr[:, 2 * oc:2 * oc + 1]
                    lo_ap = t_thr[:, 2 * oc + 1:2 * oc + 2]
                    # ternary epilogue: y = (raw > hi) - (raw < lo)
                    b = outp.tile([C, NT], dt.float32, tag="b")
                    nc.vector.tensor_scalar(b[:], pt, lo_ap, None,
                                            mybir.AluOpType.is_lt)
                    y = ypool.tile([C, NT], out_dt, tag="y")
                    nc.vector.scalar_tensor_tensor(
                        y[:], pt, hi_ap, b[:],
                        mybir.AluOpType.is_gt, mybir.AluOpType.subtract)
                if jpad:
                    if a2:
                        tix = (n * 2 + oc) * NBLK + j
                        out_eng = nc.scalar if (tix % 2) else nc.sync
                    else:
                        out_eng = nc.scalar if act_out else nc.sync
                    if dmaless:
                        out_eng.dma_start(out_d[n, oc, j][:, :8], y[:, :8])
                    else:
                        out_eng.dma_start(out_d[n, oc, j], y[:])
                else:
                    y_r = y[:].rearrange("p (r w) -> p r w", w=WP)[:, :, :W]
                    nc.sync.dma_start(out_d[oc, :, n, j * RB:j * RB + RB, :], y_r)

            jgroups = [(0, 1), (2, 3), (4, 5), (6,)] if il2 else \
                      [(j,) for j in range(NBLK)]
            for n in range(NPC):
                for oc in range(2):
                    for jg in jgroups:
                        group = []
                        for j in jg:
                            pt = psum.tile([C, 512], dt.float32, tag="pt")
                            group.append((n, oc, j, pt[:, :NT]))
                        emit_mms(group)
                        for (n_, oc_, j_, pt) in group:
                            epilogue(n_, oc_, j_, pt)

        if repeat == 1:
            body()
        else:
            with tc.For_i(0, repeat, 1):
                body()

    nc.compile()
    return nc, np_mm


def _host_prep(x, weight, bias, mode=None):
    mode = mode or MODE
    flags = set(mode.split("+")[1:])
    scale = np.clip(np.mean(np.abs(weight), axis=(1, 2, 3)), 1e-8, None)  # [O]
    sw = np.sign(weight)                                                  # [O,C,3,3]
    if "ae" in flags:
        # ACT epilogue: y2 = Sign(s*S + (b-0.5)) + Sign(s*S + (b+0.5))
        s32 = scale.astype(np.float32)
        bm = (bias - 0.5).astype(np.float32)
        bp = (bias + 0.5).astype(np.float32)
        thr = np.zeros((C, 8), dtype=np.float32)
        for oc in range(2):
            sl = slice(oc * C, (oc + 1) * C)
            thr[:, 3 * oc + 0] = s32[sl]
            thr[:, 3 * oc + 1] = bm[sl]
            thr[:, 3 * oc + 2] = bp[sl]
    else:
        hi = ((0.5 - bias.astype(np.float64)) / scale.astype(np.float64)).astype(np.float32)
        lo = ((-0.5 - bias.astype(np.float64)) / scale.astype(np.float64)).astype(np.float32)
        thr = np.stack([hi[:C], lo[:C], hi[C:], lo[C:]], axis=1).astype(np.float32)
    if "sb" in flags:
        # lhsT layout: sw[c, oc*9*128 + t*128 + o']  (oc-major, contiguous halves)
        swt = np.ascontiguousarray(
            sw.transpose(1, 2, 3, 0).reshape(C, 9, 2, C)
            .transpose(0, 2, 1, 3).reshape(C, 9 * O))
    else:
        # lhsT layout: sw[c, t*O + o]
        swt = np.ascontiguousarray(sw.transpose(1, 2, 3, 0).reshape(C, 9 * O))
    # pad x to 58x58 and lay out [C, n*3364 + hp*58 + wp]
    xp = np.zeros((N, C, HP, WP), dtype=np.float32)
    xp[:, :, 1:-1, 1:-1] = x
    xp = xp.transpose(1, 0, 2, 3).reshape(C, N * IMG)
    return thr, swt, xp


def _make_in_maps(mode, thr, swt, xp):
    pair = mode.startswith("f16p")
    in_maps = []
    for c in range(NCORES):
        xc = np.zeros((C, XCOLS_PAD), dtype=np.float32)
        xc[:, :XCOLS] = xp[:, c * XCOLS:(c + 1) * XCOLS]
        m = {"thr": thr}
        if pair:
            xh = xc.astype(np.float16)
            m["xh"] = xh
            m["xl"] = (xc - xh.astype(np.float32)).astype(np.float16)
            m["sw"] = swt.astype(np.float16)
        else:
            m["xh"] = xc
            m["sw"] = swt.copy()
        in_maps.append(m)
    return in_maps


def kernel(x, weight, bias):
    from concourse.bass_utils import run_bass_kernel_spmd

    x = np.asarray(x, dtype=np.float32)
    weight = np.asarray(weight, dtype=np.float32)
    bias = np.asarray(bias, dtype=np.float32)

    thr, swt, xp = _host_prep(x, weight, bias)

    mode = MODE
    if mode not in _prog_cache:
        _prog_cache[mode] = _build(mode)
    nc, _ = _prog_cache[mode]

    in_maps = _make_in_maps(mode, thr, swt, xp)
    res = run_bass_kernel_spmd(nc, in_maps, list(range(NCORES)))

    # ---- gather per-core outputs -> [N, O, H, W] fp32 ----
    out = np.empty((N, O, H, W), dtype=np.float32)
    for c in range(NCORES):
        oc_out = res.results[c]["out"]
        if "+j" in mode:
            # [NPC, 2, NBLK, C, NT]: rows of 58, valid w < 56
            v = np.asarray(oc_out).astype(np.float32, copy=False)
            if "+ae" in mode:
                if "+eb" in mode:
                    # only even tiles used the ACT {-2,0,2} path
                    tix = (np.arange(NPC)[:, None, None] * 2
                           + np.arange(2)[None, :, None]) * NBLK \
                        + np.arange(NBLK)[None, None, :]
                    sc = np.where(tix % 2 == 0, np.float32(0.5), np.float32(1.0))
                    v = v * sc[:, :, :, None, None]
                else:
                    v = v * np.float32(0.5)   # {-2,0,2} -> {-1,0,1}
            v = v.reshape(NPC, 2, NBLK, C, RB, WP)[:, :, :, :, :, :W]
            v = v.transpose(0, 1, 3, 2, 4, 5).reshape(NPC, O, H, W)
            out[c * NPC:(c + 1) * NPC] = v
        else:
            for oc in range(2):
                out[c * NPC:(c + 1) * NPC, oc * C:(oc + 1) * C] = \
                    oc_out[oc].transpose(1, 0, 2, 3)
    return out



# revision 37
# speedup vs baseline: 1.3695x; 1.3695x over previous
"""BinaryWeightConv2d on Trainium2 — 8-core data-parallel over batch.

Reference computation (fp32):
    scale = clip(mean|w| over (in,kh,kw), 1e-8)          # per out-channel
    bw    = sign(w) * scale
    out   = conv2d(x, bw, stride 1, pad 1) + bias
    y     = ternary(out): 1 if out > 0.5, -1 if out < -0.5, else 0

Kernel strategy (mode f32r+j+ys+sb+ae+sp2+y8+wu, ~110-130us on 8 cores vs
357us fp16-pair baseline; cool-state pairs repeatedly measure 107-115us,
matching TimelineSim's 109.8us prediction):
  - Shard the batch (32) over 8 cores, 4 images each; replicate the tiny
    binarized weights (per the data-parallel sharding hint).
  - Device: conv = 9 shifted-window matmuls (3x3 taps) accumulating in PSUM;
    contraction over C=128 = the partition dim.  x is host-padded to 58x58
    per image so every tap window is one contiguous SBUF slice.  PSUM tile =
    8 output rows x 58 = 464 columns (one bank).
  - Matmul dtype float32r: 1 PE cycle/column when free dim >= 256 (same rate
    as fp16) at ~10-bit mantissa precision, so 9 matmuls/tile instead of the
    fp16 hi/lo pair's 18.  Ternary rel err 8.7e-3 (deterministic, gate 2e-2).
    PE streaming floor 504 MMs x 464 cols ~ 97us/core; measured ~206ns/MM
    incl. the per-MM LDWEIGHTS walrus always emits (no dedup - verified in
    the sunda disasm; grouped weight reuse does NOT help, measured).
  - Epilogue on the otherwise-idle ACT engine ("ae"): per-out-channel scale s
    and biases b-+0.5 folded into two Sign activations,
    y2 = Sign(s*S + b - 0.5) + Sign(s*S + b + 0.5) in {-2,0,2};
    one DVE add combines them; host multiplies by 0.5.  Frees the DVE
    (was 51% busy) and shortens the post-matmul tail.
  - "sb": fine-grained leading DMA chunks (w 3 taps -> 6 -> rest on the ACT
    HWDGE ring; x rows 0-9 first on the sync ring) so the first matmul
    starts ~2us in instead of ~11us.
  - "sp2": the last tile's epilogue runs as two half-tiles to shorten the
    final ACT->DVE->DMA chain.
  - "y8": y stored as fp8e4 (+-2,0 exact) halving output DMA volume.
  - "wu": 64 tiny const matmuls during the leading DMA wait keep the PE busy
    so the HAM clock gate un-throttles (1.2 -> 2.4 GHz) before real work.
  - Outputs are full padded [C, 464] tiles (contiguous DMA segments); host
    strips the 2 junk columns per 58-wide row and scales by 0.5.
  - Timing method (test.py): wall-clock differencing of For_i(R) repeats at
    R=64 vs 4096, wall-floor estimator over 5 cool-ish pairs (30s idles —
    sustained load triggers a P0 downclock that inflates per-iter time ~30%).
"""

import os
import numpy as np

N, C, H, W = 32, 128, 56, 56
O = 256
NCORES = 8
NPC = N // NCORES           # images per core
HP, WP = H + 2, W + 2       # padded spatial
IMG = HP * WP               # 3364
XCOLS = NPC * IMG           # 13456
XCOLS_PAD = XCOLS + 64      # slack: the last tap of the last tile overreads 1
RB = 8                      # output rows per PSUM tile
NT = RB * WP                # 464 = PSUM tile free size (<= 512 bank limit)
NBLK = H // RB              # 7 row blocks
TAPS = [(kh, kw) for kh in range(3) for kw in range(3)]

MODE = os.environ.get("BWC_MODE", "f32r+j+ys+sb+ae+sp2+y8+wu")

_prog_cache = {}


def _build(mode, repeat=1):
    import concourse.tile as tile
    from concourse import mybir, bacc
    from contextlib import ExitStack

    dt = mybir.dt
    nc = bacc.Bacc()

    parts = mode.split("+")
    base, flags = parts[0], set(parts[1:])
    jpad = "j" in flags
    ydt_bf = "h" in flags       # store ternary output as bf16 (host converts)
    dmaless = "dl" in flags     # timing probe: ~zero output DMA volume
    act_out = "a" in flags      # issue output stores on the ACT HWDGE ring
    obufs = 16 if "o16" in flags else 6
    ysplit = "ys" in flags      # dedicated deep pool for DMA-held y tiles
    sb = "sb" in flags          # fine-grained start chunks + per-oc w layout
    ae = "ae" in flags          # ACT Sign-pair epilogue ({-2,0,2} out, host /2)
    il2 = "il2" in flags        # interleave matmuls across pairs of tiles
    sp2 = "sp2" in flags        # split last tile's epilogue into halves (tail)
    y8 = "y8" in flags          # store y as fp8e4 (exact for {-2,0,2})
    a2 = "a2" in flags          # alternate y stores across SP/ACT HWDGE rings
    eb = "eb" in flags          # balance epilogue: alternate ACT-pair / DVE-pair
    wu = "wu" in flags          # HAM warm-up: dummy matmuls during DMA wait
    # NOTE: a strided rhs AP ([8x56 stride 58] to skip junk cols) was tried
    # and CRASHES the device (NRT_EXEC_UNIT_UNRECOVERABLE) — matmul's moving
    # operand must be contiguous per partition. Do not retry.
    nt = NT

    if base == "f16p":
        mm_dt, np_mm = dt.float16, np.float16
    elif base == "f32r":
        mm_dt, np_mm = dt.float32r, np.float32
    elif base == "f32":
        mm_dt, np_mm = dt.float32, np.float32
    else:
        raise ValueError(mode)
    pair = base == "f16p"

    xh_d = nc.declare_dram_parameter("xh", [C, XCOLS_PAD], mm_dt, isOutput=False)
    xl_d = (nc.declare_dram_parameter("xl", [C, XCOLS_PAD], mm_dt, isOutput=False)
            if pair else None)
    sw_d = nc.declare_dram_parameter("sw", [C, 9 * O], mm_dt, isOutput=False)
    thr_d = nc.declare_dram_parameter("thr", [C, 8 if ae else 4], dt.float32,
                                      isOutput=False)
    out_dt = dt.float8e4 if y8 else (dt.bfloat16 if (ydt_bf or ae) else dt.float32)
    if jpad:
        out_d = nc.declare_dram_parameter("out", [NPC, 2, NBLK, C, nt],
                                          out_dt, isOutput=True)
    else:
        out_d = nc.declare_dram_parameter("out", [2, C, NPC, H, W],
                                          out_dt, isOutput=True)

    with tile.TileContext(nc) as tc, ExitStack() as ctx:
        inp = ctx.enter_context(tc.tile_pool(name="inp", bufs=2))
        outp = ctx.enter_context(tc.tile_pool(name="outp", bufs=4 if ysplit else obufs))
        ypool = (ctx.enter_context(tc.tile_pool(name="ypool", bufs=24))
                 if ysplit else outp)
        psum = ctx.enter_context(tc.tile_pool(name="psum", bufs=8, space="PSUM"))

        def body():
            t_w = inp.tile([C, 9 * O], mm_dt, tag="w")
            t_thr = inp.tile([C, 8 if ae else 4], dt.float32, tag="thr")
            t_xh = inp.tile([C, XCOLS_PAD], mm_dt, tag="xh")
            t_xl = (inp.tile([C, XCOLS_PAD], mm_dt, tag="xl", name="t_xl")
                    if pair else None)

            if sb:
                # fine-grained leading chunks so PE starts ~2us in;
                # w/thr ride the ACT HWDGE ring so their latency overlaps x's
                xbounds = [0, 10 * WP, 18 * WP, 34 * WP,
                           IMG, 2 * IMG, 3 * IMG, XCOLS_PAD]
                nc.sync.dma_start(t_xh[:, :xbounds[1]], xh_d[:, :xbounds[1]])
                nc.scalar.dma_start(t_w[:, :3 * C], sw_d[:, :3 * C])
                nc.scalar.dma_start(t_thr[:], thr_d[:])
                nc.sync.dma_start(t_xh[:, xbounds[1]:xbounds[2]],
                                  xh_d[:, xbounds[1]:xbounds[2]])
                nc.scalar.dma_start(t_w[:, 3 * C:9 * C], sw_d[:, 3 * C:9 * C])
                nc.scalar.dma_start(t_w[:, 9 * C:], sw_d[:, 9 * C:])
                for i in range(2, 7):
                    lo, hi = xbounds[i], xbounds[i + 1]
                    nc.sync.dma_start(t_xh[:, lo:hi], xh_d[:, lo:hi])
            else:
                nc.sync.dma_start(t_w[:], sw_d[:])
                nc.sync.dma_start(t_thr[:], thr_d[:])
                # chunked x loads (per image) so compute starts after chunk 0
                bounds = [0, IMG, 2 * IMG, 3 * IMG, XCOLS_PAD]
                for i in range(4):
                    lo, hi = bounds[i], bounds[i + 1]
                    nc.sync.dma_start(t_xh[:, lo:hi], xh_d[:, lo:hi])
                    if pair:
                        nc.sync.dma_start(t_xl[:, lo:hi], xl_d[:, lo:hi])

            if wu:
                # keep the PE busy through the leading DMA wait so HAM
                # un-throttles (1.2 -> 2.4 GHz) before the first real matmul
                zc = nc.const_aps.tensor(0.0, (C, 1), dt.float32)
                wpt = psum.tile([C, 512], dt.float32, tag="pt")
                for _ in range(64):
                    nc.tensor.matmul(wpt[:1, :1], zc, zc, start=True, stop=True)

            nmm = 18 if pair else 9

            def wslice(oc, t):
                if sb:
                    return t_w[:, oc * (9 * C) + t * C: oc * (9 * C) + (t + 1) * C]
                return t_w[:, t * O + oc * C: t * O + oc * C + C]

            def emit_mms(group):
                # group: list of (n, oc, j, pt); taps outer, tiles inner
                for t, (kh, kw) in enumerate(TAPS):
                    for (n, oc, j, pt) in group:
                        base_off = n * IMG + (j * RB + kh) * WP + kw
                        nc.tensor.matmul(pt, wslice(oc, t),
                                         t_xh[:, base_off:base_off + NT],
                                         start=(t == 0),
                                         stop=(t == 8 and not pair))
                        if pair:
                            nc.tensor.matmul(pt, wslice(oc, t),
                                             t_xl[:, base_off:base_off + NT],
                                             start=False, stop=(t == 8))

            def epilogue(n, oc, j, pt):
                use_act = ae
                if ae and eb and ((n * 2 + oc) * NBLK + j) % 2 == 1:
                    use_act = False  # DVE path for odd tiles ({-1,0,1} out)
                if use_act:
                    s_ap = t_thr[:, 3 * oc + 0:3 * oc + 1]
                    bm_ap = t_thr[:, 3 * oc + 1:3 * oc + 2]
                    bp_ap = t_thr[:, 3 * oc + 2:3 * oc + 3]
                    last = (n == NPC - 1 and oc == 1 and j == NBLK - 1)
                    if sp2 and last and jpad and not dmaless:
                        # shortest tail chain, per half-tile: the Sign runs on
                        # ACT in parallel with the DVE compare, then one STT:
                        #   y2 = Sign(s*S + b - 0.5) + 1 - 2*(S < lo)
                        # (same {-2,0,2} scale as the a0+a1 form)
                        lo_ap = t_thr[:, 6 + oc:7 + oc]
                        HNT = NT // 2
                        for h in range(2):
                            sl = slice(h * HNT, (h + 1) * HNT)
                            a0 = outp.tile([C, HNT], dt.bfloat16, tag="a0h")
                            nc.scalar.activation(
                                a0[:], pt[:, sl],
                                mybir.ActivationFunctionType.Sign,
                                bias=bm_ap, scale=s_ap)
                            b2 = outp.tile([C, HNT], dt.float32, tag="b2h")
                            nc.vector.tensor_scalar(
                                b2[:], pt[:, sl], lo_ap, 2.0,
                                mybir.AluOpType.is_lt, mybir.AluOpType.mult)
                            y = ypool.tile([C, HNT], out_dt, tag="yh")
                            nc.vector.scalar_tensor_tensor(
                                y[:], a0[:], 1.0, b2[:],
                                mybir.AluOpType.add, mybir.AluOpType.subtract)
                            nc.sync.dma_start(out_d[n, oc, j][:, sl], y[:])
                        return
                    a0 = outp.tile([C, NT], dt.bfloat16, tag="a0")
                    nc.scalar.activation(a0[:], pt,
                                         mybir.ActivationFunctionType.Sign,
                                         bias=bm_ap, scale=s_ap)
                    a1 = outp.tile([C, NT], dt.bfloat16, tag="a1")
                    nc.scalar.activation(a1[:], pt,
                                         mybir.ActivationFunctionType.Sign,
                                         bias=bp_ap, scale=s_ap)
                    y = ypool.tile([C, NT], out_dt, tag="y")
                    nc.vector.tensor_tensor(y[:], a0[:], a1[:],
                                            mybir.AluOpType.add)
                else:
                    hi_ap = t_thr[:, 2 * oc:2 * oc + 1]
                    lo_ap = t_thr[:, 2 * oc + 1:2 * oc + 2]
                    # ternary epilogue: y = (raw > hi) - (raw < lo)
                    b = outp.tile([C, NT], dt.float32, tag="b")
                    nc.vector.tensor_scalar(b[:], pt, lo_ap, None,
                                            mybir.AluOpType.is_lt)
                    y = ypool.tile([C, NT], out_dt, tag="y")
                    nc.vector.scalar_tensor_tensor(
                        y[:], pt, hi_ap, b[:],
                        mybir.AluOpType.is_gt, mybir.AluOpType.subtract)
                if jpad:
                    if a2:
                        tix = (n * 2 + oc) * NBLK + j
                        out_eng = nc.scalar if (tix % 2) else nc.sync
                    else:
                        out_eng = nc.scalar if act_out else nc.sync
                    if dmaless:
                        out_eng.dma_start(out_d[n, oc, j][:, :8], y[:, :8])
                    else:
                        out_eng.dma_start(out_d[n, oc, j], y[:])
                else:
                    y_r = y[:].rearrange("p (r w) -> p r w", w=WP)[:, :, :W]
                    nc.sync.dma_start(out_d[oc, :, n, j * RB:j * RB + RB, :], y_r)

            jgroups = [(0, 1), (2, 3), (4, 5), (6,)] if il2 else \
                      [(j,) for j in range(NBLK)]
            for n in range(NPC):
                for oc in range(2):
                    for jg in jgroups:
                        group = []
                        for j in jg:
                            pt = psum.tile([C, 512], dt.float32, tag="pt")
                            group.append((n, oc, j, pt[:, :NT]))
                        emit_mms(group)
                        for (n_, oc_, j_, pt) in group:
                            epilogue(n_, oc_, j_, pt)

        if repeat == 1:
            body()
        else:
            with tc.For_i(0, repeat, 1):
                body()

    nc.compile()
    return nc, np_mm


def _host_prep(x, weight, bias, mode=None):
    mode = mode or MODE
    flags = set(mode.split("+")[1:])
    scale = np.clip(np.mean(np.abs(weight), axis=(1, 2, 3)), 1e-8, None)  # [O]
    sw = np.sign(weight)                                                  # [O,C,3,3]
    if "ae" in flags:
        # ACT epilogue: y2 = Sign(s*S + (b-0.5)) + Sign(s*S + (b+0.5))
        s32 = scale.astype(np.float32)
        bm = (bias - 0.5).astype(np.float32)
        bp = (bias + 0.5).astype(np.float32)
        lo = ((-0.5 - bias.astype(np.float64))
              / scale.astype(np.float64)).astype(np.float32)
        thr = np.zeros((C, 8), dtype=np.float32)
        for oc in range(2):
            sl = slice(oc * C, (oc + 1) * C)
            thr[:, 3 * oc + 0] = s32[sl]
            thr[:, 3 * oc + 1] = bm[sl]
            thr[:, 3 * oc + 2] = bp[sl]
            thr[:, 6 + oc] = lo[sl]   # raw-S threshold for the sp2 tail path
    else:
        hi = ((0.5 - bias.astype(np.float64)) / scale.astype(np.float64)).astype(np.float32)
        lo = ((-0.5 - bias.astype(np.float64)) / scale.astype(np.float64)).astype(np.float32)
        thr = np.stack([hi[:C], lo[:C], hi[C:], lo[C:]], axis=1).astype(np.float32)
    if "sb" in flags:
        # lhsT layout: sw[c, oc*9*128 + t*128 + o']  (oc-major, contiguous halves)
        swt = np.ascontiguousarray(
            sw.transpose(1, 2, 3, 0).reshape(C, 9, 2, C)
            .transpose(0, 2, 1, 3).reshape(C, 9 * O))
    else:
        # lhsT layout: sw[c, t*O + o]
        swt = np.ascontiguousarray(sw.transpose(1, 2, 3, 0).reshape(C, 9 * O))
    # pad x to 58x58 and lay out [C, n*3364 + hp*58 + wp]
    xp = np.zeros((N, C, HP, WP), dtype=np.float32)
    xp[:, :, 1:-1, 1:-1] = x
    xp = xp.transpose(1, 0, 2, 3).reshape(C, N * IMG)
    return thr, swt, xp


def _make_in_maps(mode, thr, swt, xp):
    pair = mode.startswith("f16p")
    in_maps = []
    for c in range(NCORES):
        xc = np.zeros((C, XCOLS_PAD), dtype=np.float32)
        xc[:, :XCOLS] = xp[:, c * XCOLS:(c + 1) * XCOLS]
        m = {"thr": thr}
        if pair:
            xh = xc.astype(np.float16)
            m["xh"] = xh
            m["xl"] = (xc - xh.astype(np.float32)).astype(np.float16)
            m["sw"] = swt.astype(np.float16)
        else:
            m["xh"] = xc
            m["sw"] = swt.copy()
        in_maps.append(m)
    return in_maps


def kernel(x, weight, bias):
    from concourse.bass_utils import run_bass_kernel_spmd

    x = np.asarray(x, dtype=np.float32)
    weight = np.asarray(weight, dtype=np.float32)
    bias = np.asarray(bias, dtype=np.float32)

    thr, swt, xp = _host_prep(x, weight, bias)

    mode = MODE
    if mode not in _prog_cache:
        _prog_cache[mode] = _build(mode)
    nc, _ = _prog_cache[mode]

    in_maps = _make_in_maps(mode, thr, swt, xp)
    res = run_bass_kernel_spmd(nc, in_maps, list(range(NCORES)))

    # ---- gather per-core outputs -> [N, O, H, W] fp32 ----
    out = np.empty((N, O, H, W), dtype=np.float32)
    for c in range(NCORES):
        oc_out = res.results[c]["out"]
        if "+j" in mode:
            # [NPC, 2, NBLK, C, NT]: rows of 58, valid w < 56
            v = np.asarray(oc_out).astype(np.float32, copy=False)
            if "+ae" in mode:
                if "+eb" in mode:
                    # only even tiles used the ACT {-2,0,2} path
                    tix = (np.arange(NPC)[:, None, None] * 2
                           + np.arange(2)[None, :, None]) * NBLK \
                        + np.arange(NBLK)[None, None, :]
                    sc = np.where(tix % 2 == 0, np.float32(0.5), np.float32(1.0))
                    v = v * sc[:, :, :, None, None]
                else:
                    v = v * np.float32(0.5)   # {-2,0,2} -> {-1,0,1}
            v = v.reshape(NPC, 2, NBLK, C, RB, WP)[:, :, :, :, :, :W]
            v = v.transpose(0, 1, 3, 2, 4, 5).reshape(NPC, O, H, W)
            out[c * NPC:(c + 1) * NPC] = v
        else:
            for oc in range(2):
                out[c * NPC:(c + 1) * NPC, oc * C:(oc + 1) * C] = \
                    oc_out[oc].transpose(1, 0, 2, 3)
    return out

